# revision 40
# baseline (speedup 1.0000x reference)
"""Trainium2 Bass kernel for single-head causal attention with projections.

Reference computation (B=4, T=4096, D=1024, H=64):
    qh = q @ Wq; kh = k @ Wk; vh = v @ Wv          # [B,T,H]
    S  = qh @ kh.T / sqrt(H)  (causal masked)       # [B,T,T]
    out = softmax(S) @ vh                           # [B,T,H]

Sharding: 8 cores = 4 batches x 2 kv-halves (partial softmax).  Each
core projects the FULL query sequence but only half the kv positions
(interleaved in 128-position blocks: core h owns global kv chunk
2m + h), runs attention of all queries against its own kv half, and
emits a partial numerator [64, T] plus partial denominator [1, T].
The host adds the two partials of a batch and divides -- exact softmax,
no max subtraction needed, and crucially NO device collectives: the
in-kernel AllGather path costs ~21us of CC-engine init plus 4-14us per
op, which would gate attention until ~50us.

Performance structure:
  * tensor-engine work is the roofline (~139k PE cycles/core: 16
    projection chains + S/PV over the 36864-col causal area).  The
    scalar engine's exp runs at exactly half the PE column rate, so
    pure-attention stretches are ACT-bound; the schedule therefore
    spreads projection chains between the attention pairs so the PE
    always has surplus ready work and the HAM clock gate stays open.
  * S chunks are packed in pairs into 2-bank [128,1024] PSUM tiles so
    one ACTIVATE covers both, amortizing the ~290-cycle ACT overhead.
  * 128-block kv interleaving makes the chunk schedule identical on
    both cores of a batch (SPMD) with zero waste, and the causal
    boundary needs just ONE masked pack per query pair, with the
    stair patterns baked per-core into a [128,1024] table.
  * PSUM budget (8 banks): 2 rotating PV accumulators ([65,512], one
    per in-flight query pair), 2x2-bank S tiles, 2 projection banks.
  * the whole 12.4MB input is prefetched up front as 16 one-chain
    contiguous DMAs (host pre-tiles each chain) spread over the sync,
    gpsimd and scalar hw queues; chains are then pure compute.
  * qhT/khT are padded to 128 partitions (zero lower halves) so every
    matmul has uniform 128-row operands -- the PE then background-loads
    weights and attention matmuls stream at the 216ns/512-col optimum.
  * the v-head fold uses 4 small PE identity-transposes into a borrowed
    S-pool slot (event-time allocation keeps pool rotation in order).
  * exp activation table is pre-warmed and a short garbage-matmul
    primer plus fillers open the HAM clock gate during the DMA head.
  * scalar engine runs activations (plus 2 early dma issues); vector
    does copies and the mask multiplies.
  * output is stored transposed with the denominator as row 65; the
    host does the final combine+divide+transpose.

Schedule-stability note: PV lag=2 and the pair order 0..7 are stress-
tested (16/16 clean); reordering pairs or adding bulk DMAs to the
scalar queue showed intermittent data races (NaNs on kv groups 1-3),
so don't perturb the DMA issue structure without re-stressing.
"""

import numpy as np

B, T, D, H = 4, 4096, 1024, 64
DC = D // 128       # d chunks
NKVC = 16           # kv chunks per core (128 positions each)
NPAIR = 8           # query pairs of 512 columns
TQO = T             # q columns per core (full sequence)

_CACHE = {}


def _merge(base_events, attn_events):
    """Interleave two event lists proportionally."""
    nb, na = len(base_events), len(attn_events)
    if na == 0:
        return list(base_events)
    if nb == 0:
        return list(attn_events)
    out = []
    ai = 0
    acc = 0.0
    per = na / nb
    for ev in base_events:
        out.append(ev)
        acc += per
        while acc >= 1.0 and ai < na:
            out.append(attn_events[ai])
            ai += 1
            acc -= 1.0
    out.extend(attn_events[ai:])
    return out


def _zip2(a, b):
    """Strictly alternate two event lists, then leftovers."""
    out = []
    for x, y in zip(a, b):
        out.append(x)
        out.append(y)
    longer = a if len(a) > len(b) else b
    out.extend(longer[min(len(a), len(b)):])
    return out


def _build_program(use_mask):
    import concourse.bacc as bacc
    import concourse.mybir as mybir
    import concourse.tile as tile
    from concourse.masks import make_identity

    f32 = mybir.dt.float32
    bf16 = mybir.dt.bfloat16

    nc = bacc.Bacc(None, target_bir_lowering=False, debug=False,
                   num_devices=8)
    # streams are pre-tiled on the host: [chain, 128, DC, 512], fully
    # contiguous per partition so each chain is ONE cheap dma issue.
    qS = nc.declare_dram_parameter("qS", [8, 128, DC, 512], bf16,
                                   isOutput=False)
    kS = nc.declare_dram_parameter("kS", [4, 128, DC, 512], bf16,
                                   isOutput=False)
    vS = nc.declare_dram_parameter("vS", [4, 128, DC, 512], bf16,
                                   isOutput=False)
    wq = nc.declare_dram_parameter("wq", [128, DC, H], bf16, isOutput=False)
    wk = nc.declare_dram_parameter("wk", [128, DC, H], bf16, isOutput=False)
    wv = nc.declare_dram_parameter("wv", [128, DC, H], bf16, isOutput=False)
    pat = nc.declare_dram_parameter("pat", [128, 1024], bf16, isOutput=False)
    outT = nc.declare_dram_parameter("outT", [H + 1, TQO], f32,
                                     isOutput=True)
    scale = 1.0 / float(np.sqrt(H))

    # chunks attended by query pair jj (512 cols at 512*jj)
    def n_chunks(jj):
        return 2 * jj + 2 if use_mask else NKVC

    with tile.TileContext(nc) as tc:
        with (
            tc.tile_pool(name="singles", bufs=1) as singles,
            tc.tile_pool(name="stream", bufs=12) as stream,
            tc.tile_pool(name="psb", bufs=4) as psbp,
            tc.tile_pool(name="proj_ps", bufs=2, space="PSUM") as pps,
            tc.tile_pool(name="s_ps", bufs=2, space="PSUM") as sps,
            tc.tile_pool(name="pv_ps", bufs=2, space="PSUM") as pvp,
        ):
            wq_sb = singles.tile([128, DC, H], bf16, tag="wq")
            wk_sb = singles.tile([128, DC, H], bf16, tag="wk")
            wv_sb = singles.tile([128, DC, H], bf16, tag="wv")

            # qhT/khT padded to 128 partitions (lower halves zeroed) so S
            # and PV matmuls use uniform 128-row weights/rhs (lets the PE
            # background-load weights between matmuls).
            qhT = singles.tile([128, TQO], bf16, tag="qhT")
            khT = singles.tile([128, T // 2], bf16, tag="khT")
            vh1 = singles.tile([128, NKVC, H + 1], bf16, tag="vh1")
            pat_sb = singles.tile([128, 1024], bf16, tag="pat")

            wrm = singles.tile([128, 512], bf16, tag="wrm")
            ident32 = singles.tile([64, 64], f32, tag="id32")
            make_identity(nc, ident32)
            identb = singles.tile([64, 64], bf16, tag="idb")
            nc.vector.tensor_copy(identb, ident32)
            nc.gpsimd.memset(wrm, 0.5)
            nc.vector.memset(vh1[:, :, H:H + 1], 1.0)
            nc.vector.memset(khT[64:128, :], 0.0)
            nc.vector.memset(qhT[64:128, :], 0.0)

            # ---- prefetch the ENTIRE input up front, need-ordered: one
            # cheap contiguous DMA per chain, split over the sync and
            # gpsimd hw queues (parallel HBM bandwidth).  Chains then
            # never wait on fine-grained pacing, just their one tile.
            stiles = {
                (kind, idx): stream.tile([128, DC, 512], bf16, tag="qkv",
                                         bufs=16, name=f"st_{kind}{idx}")
                for kind, n in (("k", 4), ("v", 4), ("q", 8))
                for idx in range(n)
            }
            # scalar hw queue: the two earliest q chains, then the v
            # mid-groups (their PV need-times are late and forgiving;
            # sync alone can't land them in time); emitted first so the
            # issues run before any exp on the ACT engine
            for idx in (1, 2):
                nc.scalar.dma_start(out=stiles[("q", idx)],
                                    in_=qS[idx, :, :, :])
            nc.scalar.dma_start(out=stiles[("v", 1)], in_=vS[1, :, :, :])
            nc.scalar.dma_start(out=stiles[("v", 2)], in_=vS[2, :, :, :])
            nc.sync.dma_start(out=wk_sb, in_=wk[:, :, :])
            t0 = stiles[("k", 0)]
            nc.sync.dma_start(out=t0[:, 0:4, :], in_=kS[0, :, 0:4, :])
            nc.sync.dma_start(out=t0[:, 4:8, :], in_=kS[0, :, 4:8, :])
            nc.sync.dma_start(out=wv_sb, in_=wv[:, :, :])
            nc.sync.dma_start(out=stiles[("v", 0)], in_=vS[0, :, :, :])
            nc.sync.dma_start(out=wq_sb, in_=wq[:, :, :])
            for kind, idx, src in (("k", 1, kS), ("k", 2, kS),
                                   ("k", 3, kS), ("v", 3, vS)):
                nc.sync.dma_start(out=stiles[(kind, idx)],
                                  in_=src[idx, :, :, :])
            tq0 = stiles[("q", 0)]
            nc.gpsimd.dma_start(out=tq0[:, 0:4, :], in_=qS[0, :, 0:4, :])
            nc.gpsimd.dma_start(out=tq0[:, 4:8, :], in_=qS[0, :, 4:8, :])
            nc.gpsimd.dma_start(out=pat_sb, in_=pat[:, :])
            for idx in (3, 4, 5, 6, 7):
                nc.gpsimd.dma_start(out=stiles[("q", idx)],
                                    in_=qS[idx, :, :, :])

            # ---- HAM warmup: prime the PE clock gate with back-to-back
            # garbage matmuls while the DMA head is in flight; also fire
            # one tiny Exp to pull the ACT spline tables in early.
            warm_act = singles.tile([128, 16], bf16, tag="warm_act")
            for w in range(12):
                wps = sps.tile([128, 1024], f32, tag="sp", name=f"warm{w}")
                nc.tensor.matmul(wps[:, 0:512], wrm[:, :128], wrm,
                                 start=True, stop=True)
                if w == 0:
                    nc.scalar.activation(
                        warm_act, wps[:, 0:16],
                        mybir.ActivationFunctionType.Exp, scale=0.001)

            fill_n = [0]

            def filler():
                def go():
                    fill_n[0] += 1
                    f = sps.tile([128, 1024], f32, tag="sp",
                                 name=f"fill{fill_n[0]}")
                    nc.tensor.matmul(f[:, 0:512], wrm[:, :128], wrm,
                                     start=True, stop=True)
                return go

            # ---- projection chains (pure compute; streams prefetched)
            def chain(kind, idx, split_first=False):
                w_sb = {"q": wq_sb, "k": wk_sb, "v": wv_sb}[kind]
                ph = pps.tile([64, 512], f32, tag="ph",
                              name=f"ph{kind}{idx}")
                col0 = idx * 512
                t = stiles[(kind, idx)]
                evs = []

                def step(c):
                    def go():
                        for i in range(4):
                            nc.tensor.matmul(
                                ph, w_sb[:, c + i, :], t[:, c + i, :],
                                start=(c + i == 0), stop=(c + i == DC - 1))
                    return go
                evs.extend(step(c) for c in range(0, DC, 4))

                def evict():
                    if kind == "q":
                        nc.vector.tensor_copy(qhT[0:64, col0:col0 + 512],
                                              ph)
                    elif kind == "k":
                        nc.vector.tensor_copy(khT[0:64, col0:col0 + 512], ph)
                    else:
                        vtmp = stream.tile([64, 512], bf16, tag="vtmp",
                                           bufs=2)
                        nc.vector.tensor_copy(vtmp, ph)
                        # PE identity-transpose [64h, 128pos] x4 into a
                        # borrowed S-pool slot, then copy into the PV
                        # weight layout (no DMA on the fold path).
                        vtp = sps.tile([128, 4, H], bf16, tag="sp",
                                       name=f"vtp{idx}")
                        for s in range(4):
                            nc.tensor.transpose(
                                vtp[:, s, :],
                                vtmp[:, s * 128:(s + 1) * 128], identb)
                        nc.vector.tensor_copy(
                            vh1[:, 4 * idx:4 * idx + 4, 0:H], vtp)
                evs.append(evict)
                return evs

            # ---- attention -------------------------------------------
            def s_pack(jj, m, nm, box):
                """S + exp for chunks (m, m+1) of pair jj; mask the
                pack that straddles the causal diagonal."""
                def go():
                    sp = sps.tile([128, 1024], f32, tag="sp",
                                  name=f"s{jj}_{m}")
                    jc = 512 * jj
                    for i in (0, 1):
                        nc.tensor.matmul(
                            sp[:, 512 * i:512 * i + 512],
                            khT[:, (m + i) * 128:(m + i + 1) * 128],
                            qhT[:, jc:jc + 512],
                            start=True, stop=True)
                    p = psbp.tile([128, 1024], bf16, tag="p")
                    nc.scalar.activation(
                        p, sp, mybir.ActivationFunctionType.Exp,
                        scale=scale)
                    if use_mask and m + 2 == nm:
                        nc.vector.tensor_mul(p, p, pat_sb)
                    box.append(p)
                return go

            def pv_pack(jj, m, nm, pvt, box):
                def go():
                    p = box[0]
                    for i in (0, 1):
                        nc.tensor.matmul(
                            pvt[:, :512], vh1[:, m + i, :],
                            p[:, 512 * i:512 * i + 512],
                            start=(m + i == 0), stop=(m + i == nm - 1),
                            skip_group_check=True)
                return go

            def drain(jj, pvt):
                def go():
                    dsb = stream.tile([H + 1, 512], f32, tag="dsb",
                                      bufs=4)
                    nc.vector.tensor_copy(dsb, pvt)
                    for ps, pe in ((0, 33), (33, 65)):
                        nc.sync.dma_start(
                            out=outT[ps:pe, jj * 512:(jj + 1) * 512],
                            in_=dsb[ps:pe, :])
                return go

            def att_events(pairs, lag=2, fill=False):
                """Global software pipeline over all pairs: S-pack ...
                PV-pack (lagged), drain(jj) right after a pair's last
                PV.  fill=True slips a garbage matmul ahead of each
                S-pack so DMA waits never starve the HAM window."""
                evs = []
                pend = []  # (pv_event, after_event | None)

                def pop():
                    pv, after = pend.pop(0)
                    evs.append(pv)
                    if after is not None:
                        evs.append(after)

                for jj in pairs:
                    nm = n_chunks(jj)
                    pvt = pvp.tile([H + 1, 512], f32, tag="pv",
                                   name=f"pvt{jj}")
                    for m in range(0, nm, 2):
                        if fill:
                            evs.append(filler())
                        box = []
                        evs.append(s_pack(jj, m, nm, box))
                        last = (m + 2 == nm)
                        pend.append((pv_pack(jj, m, nm, pvt, box),
                                     drain(jj, pvt) if last else None))
                        if len(pend) > lag:
                            pop()
                while pend:
                    pop()
                return evs

            def with_fill(evs, n):
                return _merge(evs, [filler() for _ in range(n)])

            if use_mask:
                # head: kv group 0 + q pair 0, with fillers bridging the
                # DMA-latency head; then pairs with the remaining chains
                # merged in so the PE never runs dry.  Fillers between
                # chain steps bound every DMA wait to < the ~3.4us HAM
                # window so the clock gate stays open.
                head = (with_fill(_zip2(chain("k", 0, split_first=True),
                                        chain("v", 0)), 5)
                        + with_fill(chain("q", 0), 3))
                segs = [
                    ([0], with_fill(chain("q", 1), 4)),
                    ([1], with_fill(chain("k", 1) + chain("v", 1)
                                    + chain("q", 2), 6)),
                    ([2], with_fill(chain("q", 3), 3)),
                    ([3], with_fill(chain("k", 2) + chain("v", 2)
                                    + chain("q", 4), 4)),
                    ([4], chain("q", 5)),
                    ([5], chain("k", 3) + chain("v", 3) + chain("q", 6)),
                    ([6], chain("q", 7)),
                    ([7], []),
                ]
            else:
                head = (with_fill(_zip2(chain("k", 0, split_first=True),
                                        chain("v", 0)), 6)
                        + with_fill(_zip2(chain("k", 1) + chain("k", 2)
                                          + chain("k", 3),
                                          chain("v", 1) + chain("v", 2)
                                          + chain("v", 3)), 12)
                        + chain("q", 0))
                segs = [([jj], chain("q", jj + 1) if jj < 7 else [])
                        for jj in range(NPAIR)]

            for ev in head:
                ev()
            for pairs, chains in segs:
                for ev in _merge(chains, att_events(pairs)):
                    ev()
    nc.compile()
    return nc


def _get_program(key, use_mask):
    if key not in _CACHE:
        _CACHE[key] = _build_program(use_mask)
    return _CACHE[key]


def _numpy_fallback(q, k, v, mask, Wq, Wk, Wv):
    qh = q.astype(np.float32) @ Wq
    kh = k.astype(np.float32) @ Wk
    vh = v.astype(np.float32) @ Wv
    out = np.empty((B, T, H), np.float32)
    neg = np.float32(-1e30)
    for b in range(B):
        s = (qh[b] @ kh[b].T) / np.float32(np.sqrt(H))
        s = np.where(mask == 0, neg, s)
        s = s - s.max(axis=-1, keepdims=True)
        e = np.exp(s)
        w = e / e.sum(axis=-1, keepdims=True)
        out[b] = w @ vh[b]
    return out


def _w_layout(w, np_in):
    """[D, H] -> [128, DC, H]: partition-major layout for dense DMA."""
    return np.ascontiguousarray(
        w.reshape(DC, 128, H).transpose(1, 0, 2), np_in)


def _make_pat(half):
    """[128, 1024] stair mask for the diagonal pack: chunk m=2jj+h*?
    block i covers kv offsets 128*half + 256*i relative to the pair's
    first query column."""
    p = np.arange(128)[:, None]
    c = np.arange(512)[None, :]
    blocks = [(c >= 128 * half + 256 * i + p).astype(np.float32)
              for i in (0, 1)]
    return np.concatenate(blocks, axis=1)  # [128, 1024]


def _stream_pack(slabT, np_in):
    """[D, ncols] -> [ncols//512, 128, DC, 512] chain-major stream."""
    ncols = slabT.shape[1]
    return np.ascontiguousarray(
        slabT.reshape(DC, 128, ncols // 512, 512).transpose(2, 1, 0, 3),
        np_in)


def _make_in_maps(q, k, v, Wq, Wk, Wv, apply_tail, np_in):
    # core h owns global kv chunks 2m + h (128-position blocks)
    idx = [
        np.concatenate([np.arange(128 * (2 * m + h), 128 * (2 * m + h) + 128)
                        for m in range(NKVC)])
        for h in range(2)
    ]
    ones = np.ones((128, 1024), np.float32)
    in_maps = []
    for core in range(8):
        b, h = divmod(core, 2)
        im = {
            "qS": _stream_pack(q[b].T, np_in),
            "kS": _stream_pack(k[b].T[:, idx[h]], np_in),
            "vS": _stream_pack(v[b].T[:, idx[h]], np_in),
            "wq": _w_layout(Wq, np_in), "wk": _w_layout(Wk, np_in),
            "wv": _w_layout(Wv, np_in),
            "pat": np.ascontiguousarray(
                _make_pat(h) if apply_tail else ones, np_in),
        }
        in_maps.append(im)
    return in_maps


def kernel(q, k, v, mask, Wq, Wk, Wv):
    from concourse.bass_utils import run_bass_kernel_spmd
    import ml_dtypes

    q = np.ascontiguousarray(q, np.float32)
    k = np.ascontiguousarray(k, np.float32)
    v = np.ascontiguousarray(v, np.float32)
    Wq = np.ascontiguousarray(Wq, np.float32)
    Wk = np.ascontiguousarray(Wk, np.float32)
    Wv = np.ascontiguousarray(Wv, np.float32)
    mask = np.asarray(mask)

    is_tril = bool((mask == np.tril(np.ones((T, T), mask.dtype))).all())
    if not is_tril:
        return _numpy_fallback(q, k, v, mask, Wq, Wk, Wv)

    np_in = ml_dtypes.bfloat16
    nc = _get_program(("v26", is_tril), is_tril)

    in_maps = _make_in_maps(q, k, v, Wq, Wk, Wv, is_tril, np_in)
    res = run_bass_kernel_spmd(nc, in_maps, list(range(8)))

    out = np.empty((B, T, H), np.float32)
    for b in range(B):
        o0 = res.results[2 * b]["outT"]      # [H+1, T] partials
        o1 = res.results[2 * b + 1]["outT"]
        num = o0[:H, :] + o1[:H, :]
        den = o0[H:H + 1, :] + o1[H:H + 1, :]
        out[b] = (num / den).T
    return out


# revision 41
# speedup vs baseline: 1.0509x; 1.0509x over previous
"""Trainium2 Bass kernel for single-head causal attention with projections.

Reference computation (B=4, T=4096, D=1024, H=64):
    qh = q @ Wq; kh = k @ Wk; vh = v @ Wv          # [B,T,H]
    S  = qh @ kh.T / sqrt(H)  (causal masked)       # [B,T,T]
    out = softmax(S) @ vh                           # [B,T,H]

Sharding: 8 cores = 4 batches x 2 kv-halves (partial softmax).  Each
core projects the FULL query sequence but only half the kv positions
(interleaved in 128-position blocks: core h owns global kv chunk
2m + h), runs attention of all queries against its own kv half, and
emits a partial numerator [64, T] plus partial denominator [1, T].
The host adds the two partials of a batch and divides -- exact softmax,
no max subtraction needed, and crucially NO device collectives: the
in-kernel AllGather path costs ~21us of CC-engine init plus 4-14us per
op, which would gate attention until ~50us.

Performance structure:
  * tensor-engine work is the roofline (~139k PE cycles/core: 16
    projection chains + S/PV over the 36864-col causal area).  The
    scalar engine's exp runs at exactly half the PE column rate, so
    pure-attention stretches are ACT-bound; the schedule therefore
    spreads projection chains between the attention pairs so the PE
    always has surplus ready work and the HAM clock gate stays open.
  * S chunks are packed in pairs into 2-bank [128,1024] PSUM tiles so
    one ACTIVATE covers both, amortizing the ~290-cycle ACT overhead.
  * 128-block kv interleaving makes the chunk schedule identical on
    both cores of a batch (SPMD) with zero waste, and the causal
    boundary needs just ONE masked pack per query pair, with the
    stair patterns baked per-core into a [128,1024] table.
  * PSUM budget (8 banks): 2 rotating PV accumulators ([65,512], one
    per in-flight query pair), 2x2-bank S tiles, 2 projection banks.
  * the whole 12.4MB input is prefetched up front as 16 one-chain
    contiguous DMAs (host pre-tiles each chain) spread over the sync,
    gpsimd and scalar hw queues; chains are then pure compute.
  * qhT/khT are padded to 128 partitions (zero lower halves) so every
    matmul has uniform 128-row operands -- the PE then background-loads
    weights and attention matmuls stream at the 216ns/512-col optimum.
  * the v-head fold uses 4 small PE identity-transposes into a borrowed
    S-pool slot (event-time allocation keeps pool rotation in order).
  * exp activation table is pre-warmed and a short garbage-matmul
    primer plus fillers open the HAM clock gate during the DMA head.
  * scalar engine runs activations (plus 2 early dma issues); vector
    does copies and the mask multiplies.
  * output is stored transposed with the denominator as row 65; the
    host does the final combine+divide+transpose.

Schedule-stability note: PV lag=2 and the pair order 0..7 are stress-
tested (16/16 clean); reordering pairs or adding bulk DMAs to the
scalar queue showed intermittent data races (NaNs on kv groups 1-3),
so don't perturb the DMA issue structure without re-stressing.
"""

import numpy as np

B, T, D, H = 4, 4096, 1024, 64
DC = D // 128       # d chunks
NKVC = 16           # kv chunks per core (128 positions each)
NPAIR = 8           # query pairs of 512 columns
TQO = T             # q columns per core (full sequence)

_CACHE = {}


def _merge(base_events, attn_events):
    """Interleave two event lists proportionally."""
    nb, na = len(base_events), len(attn_events)
    if na == 0:
        return list(base_events)
    if nb == 0:
        return list(attn_events)
    out = []
    ai = 0
    acc = 0.0
    per = na / nb
    for ev in base_events:
        out.append(ev)
        acc += per
        while acc >= 1.0 and ai < na:
            out.append(attn_events[ai])
            ai += 1
            acc -= 1.0
    out.extend(attn_events[ai:])
    return out


def _zip2(a, b):
    """Strictly alternate two event lists, then leftovers."""
    out = []
    for x, y in zip(a, b):
        out.append(x)
        out.append(y)
    longer = a if len(a) > len(b) else b
    out.extend(longer[min(len(a), len(b)):])
    return out


def _build_program(use_mask):
    import concourse.bacc as bacc
    import concourse.mybir as mybir
    import concourse.tile as tile
    from concourse.masks import make_identity

    f32 = mybir.dt.float32
    bf16 = mybir.dt.bfloat16

    nc = bacc.Bacc(None, target_bir_lowering=False, debug=False,
                   num_devices=8)
    # streams are pre-tiled on the host: [chain, 128, DC, 512], fully
    # contiguous per partition so each chain is ONE cheap dma issue.
    qS = nc.declare_dram_parameter("qS", [8, 128, DC, 512], bf16,
                                   isOutput=False)
    kS = nc.declare_dram_parameter("kS", [4, 128, DC, 512], bf16,
                                   isOutput=False)
    vS = nc.declare_dram_parameter("vS", [4, 128, DC, 512], bf16,
                                   isOutput=False)
    wq = nc.declare_dram_parameter("wq", [128, DC, H], bf16, isOutput=False)
    wk = nc.declare_dram_parameter("wk", [128, DC, H], bf16, isOutput=False)
    wv = nc.declare_dram_parameter("wv", [128, DC, H], bf16, isOutput=False)
    pat = nc.declare_dram_parameter("pat", [128, 1024], bf16, isOutput=False)
    outT = nc.declare_dram_parameter("outT", [H + 1, TQO], f32,
                                     isOutput=True)
    scale = 1.0 / float(np.sqrt(H))

    # chunks attended by query pair jj (512 cols at 512*jj)
    def n_chunks(jj):
        return 2 * jj + 2 if use_mask else NKVC

    with tile.TileContext(nc) as tc:
        with (
            tc.tile_pool(name="singles", bufs=1) as singles,
            tc.tile_pool(name="stream", bufs=12) as stream,
            tc.tile_pool(name="psb", bufs=4) as psbp,
            tc.tile_pool(name="proj_ps", bufs=2, space="PSUM") as pps,
            tc.tile_pool(name="s_ps", bufs=2, space="PSUM") as sps,
            tc.tile_pool(name="pv_ps", bufs=2, space="PSUM") as pvp,
        ):
            wq_sb = singles.tile([128, DC, H], bf16, tag="wq")
            wk_sb = singles.tile([128, DC, H], bf16, tag="wk")
            wv_sb = singles.tile([128, DC, H], bf16, tag="wv")

            # qhT/khT padded to 128 partitions (lower halves zeroed) so S
            # and PV matmuls use uniform 128-row weights/rhs (lets the PE
            # background-load weights between matmuls).
            qhT = singles.tile([128, TQO], bf16, tag="qhT")
            khT = singles.tile([128, T // 2], bf16, tag="khT")
            vh1 = singles.tile([128, NKVC, H + 1], bf16, tag="vh1")
            pat_sb = singles.tile([128, 1024], bf16, tag="pat")

            wrm = singles.tile([128, 512], bf16, tag="wrm")
            ident32 = singles.tile([64, 64], f32, tag="id32")
            make_identity(nc, ident32)
            identb = singles.tile([64, 64], bf16, tag="idb")
            nc.vector.tensor_copy(identb, ident32)
            nc.gpsimd.memset(wrm, 0.5)
            nc.vector.memset(vh1[:, :, H:H + 1], 1.0)
            nc.vector.memset(khT[64:128, :], 0.0)
            nc.vector.memset(qhT[64:128, :], 0.0)

            # ---- prefetch the ENTIRE input up front, need-ordered: one
            # cheap contiguous DMA per chain, split over the sync and
            # gpsimd hw queues (parallel HBM bandwidth).  Chains then
            # never wait on fine-grained pacing, just their one tile.
            stiles = {
                (kind, idx): stream.tile([128, DC, 512], bf16, tag="qkv",
                                         bufs=16, name=f"st_{kind}{idx}")
                for kind, n in (("k", 4), ("v", 4), ("q", 8))
                for idx in range(n)
            }
            # scalar hw queue: the two earliest q chains, emitted before
            # anything else so their issues run first on the ACT engine
            for idx in (1, 2):
                nc.scalar.dma_start(out=stiles[("q", idx)],
                                    in_=qS[idx, :, :, :])
            nc.sync.dma_start(out=wk_sb, in_=wk[:, :, :])
            t0 = stiles[("k", 0)]
            nc.sync.dma_start(out=t0[:, 0:4, :], in_=kS[0, :, 0:4, :])
            nc.sync.dma_start(out=t0[:, 4:8, :], in_=kS[0, :, 4:8, :])
            nc.sync.dma_start(out=wv_sb, in_=wv[:, :, :])
            nc.sync.dma_start(out=stiles[("v", 0)], in_=vS[0, :, :, :])
            nc.sync.dma_start(out=wq_sb, in_=wq[:, :, :])
            for kind, idx, src in (("k", 1, kS), ("v", 1, vS),
                                   ("k", 2, kS), ("v", 2, vS),
                                   ("k", 3, kS), ("v", 3, vS)):
                nc.sync.dma_start(out=stiles[(kind, idx)],
                                  in_=src[idx, :, :, :])
            tq0 = stiles[("q", 0)]
            nc.gpsimd.dma_start(out=tq0[:, 0:4, :], in_=qS[0, :, 0:4, :])
            nc.gpsimd.dma_start(out=tq0[:, 4:8, :], in_=qS[0, :, 4:8, :])
            nc.gpsimd.dma_start(out=pat_sb, in_=pat[:, :])
            for idx in (3, 4, 5, 6, 7):
                nc.gpsimd.dma_start(out=stiles[("q", idx)],
                                    in_=qS[idx, :, :, :])

            # ---- HAM warmup: prime the PE clock gate with back-to-back
            # garbage matmuls while the DMA head is in flight; also fire
            # one tiny Exp to pull the ACT spline tables in early.
            warm_act = singles.tile([128, 16], bf16, tag="warm_act")
            for w in range(12):
                wps = sps.tile([128, 1024], f32, tag="sp", name=f"warm{w}")
                nc.tensor.matmul(wps[:, 0:512], wrm[:, :128], wrm,
                                 start=True, stop=True)
                if w == 0:
                    nc.scalar.activation(
                        warm_act, wps[:, 0:16],
                        mybir.ActivationFunctionType.Exp, scale=0.001)

            fill_n = [0]

            def filler():
                def go():
                    fill_n[0] += 1
                    f = sps.tile([128, 1024], f32, tag="sp",
                                 name=f"fill{fill_n[0]}")
                    nc.tensor.matmul(f[:, 0:512], wrm[:, :128], wrm,
                                     start=True, stop=True)
                return go

            # ---- projection chains (pure compute; streams prefetched)
            def chain(kind, idx, split_first=False):
                w_sb = {"q": wq_sb, "k": wk_sb, "v": wv_sb}[kind]
                ph = pps.tile([64, 512], f32, tag="ph",
                              name=f"ph{kind}{idx}")
                col0 = idx * 512
                t = stiles[(kind, idx)]
                evs = []

                def step(c):
                    def go():
                        for i in range(4):
                            nc.tensor.matmul(
                                ph, w_sb[:, c + i, :], t[:, c + i, :],
                                start=(c + i == 0), stop=(c + i == DC - 1))
                    return go
                evs.extend(step(c) for c in range(0, DC, 4))

                def evict():
                    if kind == "q":
                        nc.vector.tensor_copy(qhT[0:64, col0:col0 + 512],
                                              ph)
                    elif kind == "k":
                        nc.vector.tensor_copy(khT[0:64, col0:col0 + 512], ph)
                    else:
                        vtmp = stream.tile([64, 512], bf16, tag="vtmp",
                                           bufs=2)
                        nc.vector.tensor_copy(vtmp, ph)
                        # PE identity-transpose [64h, 128pos] x4 into a
                        # borrowed S-pool slot, then copy into the PV
                        # weight layout (no DMA on the fold path).
                        vtp = sps.tile([128, 4, H], bf16, tag="sp",
                                       name=f"vtp{idx}")
                        for s in range(4):
                            nc.tensor.transpose(
                                vtp[:, s, :],
                                vtmp[:, s * 128:(s + 1) * 128], identb)
                        nc.vector.tensor_copy(
                            vh1[:, 4 * idx:4 * idx + 4, 0:H], vtp)
                evs.append(evict)
                return evs

            # ---- attention -------------------------------------------
            def s_pack(jj, m, nm, box):
                """S + exp for chunks (m, m+1) of pair jj; mask the
                pack that straddles the causal diagonal."""
                def go():
                    sp = sps.tile([128, 1024], f32, tag="sp",
                                  name=f"s{jj}_{m}")
                    jc = 512 * jj
                    for i in (0, 1):
                        nc.tensor.matmul(
                            sp[:, 512 * i:512 * i + 512],
                            khT[:, (m + i) * 128:(m + i + 1) * 128],
                            qhT[:, jc:jc + 512],
                            start=True, stop=True)
                    p = psbp.tile([128, 1024], bf16, tag="p")
                    nc.scalar.activation(
                        p, sp, mybir.ActivationFunctionType.Exp,
                        scale=scale)
                    if use_mask and m + 2 == nm:
                        nc.vector.tensor_mul(p, p, pat_sb)
                    box.append(p)
                return go

            def pv_pack(jj, m, nm, pvt, box):
                def go():
                    p = box[0]
                    for i in (0, 1):
                        nc.tensor.matmul(
                            pvt[:, :512], vh1[:, m + i, :],
                            p[:, 512 * i:512 * i + 512],
                            start=(m + i == 0), stop=(m + i == nm - 1),
                            skip_group_check=True)
                return go

            def drain(jj, pvt):
                def go():
                    dsb = stream.tile([H + 1, 512], f32, tag="dsb",
                                      bufs=4)
                    nc.vector.tensor_copy(dsb, pvt)
                    for ps, pe in ((0, 33), (33, 65)):
                        nc.sync.dma_start(
                            out=outT[ps:pe, jj * 512:(jj + 1) * 512],
                            in_=dsb[ps:pe, :])
                return go

            def att_events(pairs, lag=2, fill=False):
                """Global software pipeline over all pairs: S-pack ...
                PV-pack (lagged), drain(jj) right after a pair's last
                PV.  fill=True slips a garbage matmul ahead of each
                S-pack so DMA waits never starve the HAM window."""
                evs = []
                pend = []  # (pv_event, after_event | None)

                def pop():
                    pv, after = pend.pop(0)
                    evs.append(pv)
                    if after is not None:
                        evs.append(after)

                for jj in pairs:
                    nm = n_chunks(jj)
                    pvt = pvp.tile([H + 1, 512], f32, tag="pv",
                                   name=f"pvt{jj}")
                    for m in range(0, nm, 2):
                        if fill:
                            evs.append(filler())
                        box = []
                        evs.append(s_pack(jj, m, nm, box))
                        last = (m + 2 == nm)
                        pend.append((pv_pack(jj, m, nm, pvt, box),
                                     drain(jj, pvt) if last else None))
                        if len(pend) > lag:
                            pop()
                while pend:
                    pop()
                return evs

            def with_fill(evs, n):
                return _merge(evs, [filler() for _ in range(n)])

            if use_mask:
                # head: kv group 0 + q pair 0, with fillers bridging the
                # DMA-latency head; then pairs with the remaining chains
                # merged in so the PE never runs dry.  Fillers between
                # chain steps bound every DMA wait to < the ~3.4us HAM
                # window so the clock gate stays open.
                head = (with_fill(_zip2(chain("k", 0, split_first=True),
                                        chain("v", 0)), 5)
                        + with_fill(chain("q", 0), 3))
                segs = [
                    ([0], with_fill(chain("q", 1), 4)),
                    ([1], with_fill(chain("k", 1) + chain("v", 1)
                                    + chain("q", 2), 6)),
                    ([2], with_fill(chain("q", 3), 3)),
                    ([3], with_fill(chain("k", 2) + chain("v", 2)
                                    + chain("q", 4), 4)),
                    ([4], chain("q", 5)),
                    ([5], chain("k", 3) + chain("v", 3) + chain("q", 6)),
                    ([6], chain("q", 7)),
                    ([7], []),
                ]
            else:
                head = (with_fill(_zip2(chain("k", 0, split_first=True),
                                        chain("v", 0)), 6)
                        + with_fill(_zip2(chain("k", 1) + chain("k", 2)
                                          + chain("k", 3),
                                          chain("v", 1) + chain("v", 2)
                                          + chain("v", 3)), 12)
                        + chain("q", 0))
                segs = [([jj], chain("q", jj + 1) if jj < 7 else [])
                        for jj in range(NPAIR)]

            for ev in head:
                ev()
            for pairs, chains in segs:
                for ev in _merge(chains, att_events(pairs)):
                    ev()
    nc.compile()
    return nc


def _get_program(key, use_mask):
    if key not in _CACHE:
        _CACHE[key] = _build_program(use_mask)
    return _CACHE[key]


def _numpy_fallback(q, k, v, mask, Wq, Wk, Wv):
    qh = q.astype(np.float32) @ Wq
    kh = k.astype(np.float32) @ Wk
    vh = v.astype(np.float32) @ Wv
    out = np.empty((B, T, H), np.float32)
    neg = np.float32(-1e30)
    for b in range(B):
        s = (qh[b] @ kh[b].T) / np.float32(np.sqrt(H))
        s = np.where(mask == 0, neg, s)
        s = s - s.max(axis=-1, keepdims=True)
        e = np.exp(s)
        w = e / e.sum(axis=-1, keepdims=True)
        out[b] = w @ vh[b]
    return out


def _w_layout(w, np_in):
    """[D, H] -> [128, DC, H]: partition-major layout for dense DMA."""
    return np.ascontiguousarray(
        w.reshape(DC, 128, H).transpose(1, 0, 2), np_in)


def _make_pat(half):
    """[128, 1024] stair mask for the diagonal pack: chunk m=2jj+h*?
    block i covers kv offsets 128*half + 256*i relative to the pair's
    first query column."""
    p = np.arange(128)[:, None]
    c = np.arange(512)[None, :]
    blocks = [(c >= 128 * half + 256 * i + p).astype(np.float32)
              for i in (0, 1)]
    return np.concatenate(blocks, axis=1)  # [128, 1024]


def _stream_pack(slabT, np_in):
    """[D, ncols] -> [ncols//512, 128, DC, 512] chain-major stream."""
    ncols = slabT.shape[1]
    return np.ascontiguousarray(
        slabT.reshape(DC, 128, ncols // 512, 512).transpose(2, 1, 0, 3),
        np_in)


def _make_in_maps(q, k, v, Wq, Wk, Wv, apply_tail, np_in):
    # core h owns global kv chunks 2m + h (128-position blocks)
    idx = [
        np.concatenate([np.arange(128 * (2 * m + h), 128 * (2 * m + h) + 128)
                        for m in range(NKVC)])
        for h in range(2)
    ]
    ones = np.ones((128, 1024), np.float32)
    in_maps = []
    for core in range(8):
        b, h = divmod(core, 2)
        im = {
            "qS": _stream_pack(q[b].T, np_in),
            "kS": _stream_pack(k[b].T[:, idx[h]], np_in),
            "vS": _stream_pack(v[b].T[:, idx[h]], np_in),
            "wq": _w_layout(Wq, np_in), "wk": _w_layout(Wk, np_in),
            "wv": _w_layout(Wv, np_in),
            "pat": np.ascontiguousarray(
                _make_pat(h) if apply_tail else ones, np_in),
        }
        in_maps.append(im)
    return in_maps


def kernel(q, k, v, mask, Wq, Wk, Wv):
    from concourse.bass_utils import run_bass_kernel_spmd
    import ml_dtypes

    q = np.ascontiguousarray(q, np.float32)
    k = np.ascontiguousarray(k, np.float32)
    v = np.ascontiguousarray(v, np.float32)
    Wq = np.ascontiguousarray(Wq, np.float32)
    Wk = np.ascontiguousarray(Wk, np.float32)
    Wv = np.ascontiguousarray(Wv, np.float32)
    mask = np.asarray(mask)

    is_tril = bool((mask == np.tril(np.ones((T, T), mask.dtype))).all())
    if not is_tril:
        return _numpy_fallback(q, k, v, mask, Wq, Wk, Wv)

    np_in = ml_dtypes.bfloat16
    nc = _get_program(("v25", is_tril), is_tril)

    in_maps = _make_in_maps(q, k, v, Wq, Wk, Wv, is_tril, np_in)
    res = run_bass_kernel_spmd(nc, in_maps, list(range(8)))

    out = np.empty((B, T, H), np.float32)
    for b in range(B):
        o0 = res.results[2 * b]["outT"]      # [H+1, T] partials
        o1 = res.results[2 * b + 1]["outT"]
        num = o0[:H, :] + o1[:H, :]
        den = o0[H:H + 1, :] + o1[H:H + 1, :]
        out[b] = (num / den).T
    return out


# revision 42
# speedup vs baseline: 1.0664x; 1.0147x over previous
"""Trainium2 Bass kernel for single-head causal attention with projections.

Reference computation (B=4, T=4096, D=1024, H=64):
    qh = q @ Wq; kh = k @ Wk; vh = v @ Wv          # [B,T,H]
    S  = qh @ kh.T / sqrt(H)  (causal masked)       # [B,T,T]
    out = softmax(S) @ vh                           # [B,T,H]

Sharding: 8 cores = 4 batches x 2 kv-halves (partial softmax).  Each
core projects the FULL query sequence but only half the kv positions
(interleaved in 128-position blocks: core h owns global kv chunk
2m + h), runs attention of all queries against its own kv half, and
emits a partial numerator [64, T] plus partial denominator [1, T].
The host adds the two partials of a batch and divides -- exact softmax,
no max subtraction needed, and crucially NO device collectives: the
in-kernel AllGather path costs ~21us of CC-engine init plus 4-14us per
op, which would gate attention until ~50us.

Performance structure:
  * tensor-engine work is the roofline (~139k PE cycles/core: 16
    projection chains + S/PV over the 36864-col causal area).  The
    scalar engine's exp runs at exactly half the PE column rate, so
    pure-attention stretches are ACT-bound; the schedule therefore
    spreads projection chains between the attention pairs so the PE
    always has surplus ready work and the HAM clock gate stays open.
  * S chunks are packed in pairs into 2-bank [128,1024] PSUM tiles so
    one ACTIVATE covers both, amortizing the ~290-cycle ACT overhead.
  * 128-block kv interleaving makes the chunk schedule identical on
    both cores of a batch (SPMD) with zero waste, and the causal
    boundary needs just ONE masked pack per query pair, with the
    stair patterns baked per-core into a [128,1024] table.
  * PSUM budget (8 banks): 2 rotating PV accumulators ([65,512], one
    per in-flight query pair), 2x2-bank S tiles, 2 projection banks.
  * the whole 12.4MB input is prefetched up front as 16 one-chain
    contiguous DMAs (host pre-tiles each chain) spread over the sync,
    gpsimd and scalar hw queues; chains are then pure compute.
  * qhT/khT are padded to 128 partitions (zero lower halves) so every
    matmul has uniform 128-row operands -- the PE then background-loads
    weights and attention matmuls stream at the 216ns/512-col optimum.
  * the v-head fold uses 4 small PE identity-transposes into a borrowed
    S-pool slot (event-time allocation keeps pool rotation in order).
  * exp activation table is pre-warmed and a short garbage-matmul
    primer plus fillers open the HAM clock gate during the DMA head.
  * scalar engine runs activations (plus 2 early dma issues); vector
    does copies and the mask multiplies.
  * output is stored transposed with the denominator as row 65; the
    host does the final combine+divide+transpose.

Schedule-stability note: PV lag=2 and the pair order 0..7 are stress-
tested (16/16 clean); reordering pairs or adding bulk DMAs to the
scalar queue showed intermittent data races (NaNs on kv groups 1-3),
so don't perturb the DMA issue structure without re-stressing.
"""

import numpy as np

B, T, D, H = 4, 4096, 1024, 64
DC = D // 128       # d chunks
NKVC = 16           # kv chunks per core (128 positions each)
NPAIR = 8           # query pairs of 512 columns
TQO = T             # q columns per core (full sequence)

_CACHE = {}


def _merge(base_events, attn_events):
    """Interleave two event lists proportionally."""
    nb, na = len(base_events), len(attn_events)
    if na == 0:
        return list(base_events)
    if nb == 0:
        return list(attn_events)
    out = []
    ai = 0
    acc = 0.0
    per = na / nb
    for ev in base_events:
        out.append(ev)
        acc += per
        while acc >= 1.0 and ai < na:
            out.append(attn_events[ai])
            ai += 1
            acc -= 1.0
    out.extend(attn_events[ai:])
    return out


def _zip2(a, b):
    """Strictly alternate two event lists, then leftovers."""
    out = []
    for x, y in zip(a, b):
        out.append(x)
        out.append(y)
    longer = a if len(a) > len(b) else b
    out.extend(longer[min(len(a), len(b)):])
    return out


def _build_program(use_mask):
    import concourse.bacc as bacc
    import concourse.mybir as mybir
    import concourse.tile as tile
    from concourse.masks import make_identity

    f32 = mybir.dt.float32
    bf16 = mybir.dt.bfloat16

    nc = bacc.Bacc(None, target_bir_lowering=False, debug=False,
                   num_devices=8)
    # streams are pre-tiled on the host: [chain, 128, DC, 512], fully
    # contiguous per partition so each chain is ONE cheap dma issue.
    qS = nc.declare_dram_parameter("qS", [8, 128, DC, 512], bf16,
                                   isOutput=False)
    kS = nc.declare_dram_parameter("kS", [4, 128, DC, 512], bf16,
                                   isOutput=False)
    vS = nc.declare_dram_parameter("vS", [4, 128, DC, 512], bf16,
                                   isOutput=False)
    wq = nc.declare_dram_parameter("wq", [128, DC, H], bf16, isOutput=False)
    wk = nc.declare_dram_parameter("wk", [128, DC, H], bf16, isOutput=False)
    wv = nc.declare_dram_parameter("wv", [128, DC, H], bf16, isOutput=False)
    pat = nc.declare_dram_parameter("pat", [128, 1024], bf16, isOutput=False)
    outT = nc.declare_dram_parameter("outT", [H + 1, TQO], bf16,
                                     isOutput=True)
    scale = 1.0 / float(np.sqrt(H))

    # chunks attended by query pair jj (512 cols at 512*jj)
    def n_chunks(jj):
        return 2 * jj + 2 if use_mask else NKVC

    with tile.TileContext(nc) as tc:
        with (
            tc.tile_pool(name="singles", bufs=1) as singles,
            tc.tile_pool(name="stream", bufs=12) as stream,
            tc.tile_pool(name="psb", bufs=4) as psbp,
            tc.tile_pool(name="proj_ps", bufs=2, space="PSUM") as pps,
            tc.tile_pool(name="s_ps", bufs=2, space="PSUM") as sps,
            tc.tile_pool(name="pv_ps", bufs=2, space="PSUM") as pvp,
        ):
            wq_sb = singles.tile([128, DC, H], bf16, tag="wq")
            wk_sb = singles.tile([128, DC, H], bf16, tag="wk")
            wv_sb = singles.tile([128, DC, H], bf16, tag="wv")

            # qhT/khT padded to 128 partitions (lower halves zeroed) so S
            # and PV matmuls use uniform 128-row weights/rhs (lets the PE
            # background-load weights between matmuls).
            qhT = singles.tile([128, TQO], bf16, tag="qhT")
            khT = singles.tile([128, T // 2], bf16, tag="khT")
            vh1 = singles.tile([128, NKVC, H + 1], bf16, tag="vh1")
            pat_sb = singles.tile([128, 1024], bf16, tag="pat")

            wrm = singles.tile([128, 512], bf16, tag="wrm")
            ident32 = singles.tile([64, 64], f32, tag="id32")
            make_identity(nc, ident32)
            identb = singles.tile([64, 64], bf16, tag="idb")
            nc.vector.tensor_copy(identb, ident32)
            nc.gpsimd.memset(wrm, 0.5)
            nc.vector.memset(vh1[:, :, H:H + 1], 1.0)
            nc.vector.memset(khT[64:128, :], 0.0)
            nc.vector.memset(qhT[64:128, :], 0.0)

            # ---- prefetch the ENTIRE input up front, need-ordered: one
            # cheap contiguous DMA per chain, split over the sync and
            # gpsimd hw queues (parallel HBM bandwidth).  Chains then
            # never wait on fine-grained pacing, just their one tile.
            stiles = {
                (kind, idx): stream.tile([128, DC, 512], bf16, tag="qkv",
                                         bufs=16, name=f"st_{kind}{idx}")
                for kind, n in (("k", 4), ("v", 4), ("q", 8))
                for idx in range(n)
            }
            # scalar hw queue: the two earliest q chains, emitted before
            # anything else so their issues run first on the ACT engine
            for idx in (1, 2):
                nc.scalar.dma_start(out=stiles[("q", idx)],
                                    in_=qS[idx, :, :, :])
            nc.sync.dma_start(out=wk_sb, in_=wk[:, :, :])
            t0 = stiles[("k", 0)]
            nc.sync.dma_start(out=t0[:, 0:4, :], in_=kS[0, :, 0:4, :])
            nc.sync.dma_start(out=t0[:, 4:8, :], in_=kS[0, :, 4:8, :])
            nc.sync.dma_start(out=wv_sb, in_=wv[:, :, :])
            nc.sync.dma_start(out=stiles[("v", 0)], in_=vS[0, :, :, :])
            nc.sync.dma_start(out=wq_sb, in_=wq[:, :, :])
            for kind, idx, src in (("k", 1, kS), ("v", 1, vS),
                                   ("k", 2, kS), ("v", 2, vS),
                                   ("k", 3, kS), ("v", 3, vS)):
                nc.sync.dma_start(out=stiles[(kind, idx)],
                                  in_=src[idx, :, :, :])
            tq0 = stiles[("q", 0)]
            nc.gpsimd.dma_start(out=tq0[:, 0:4, :], in_=qS[0, :, 0:4, :])
            nc.gpsimd.dma_start(out=tq0[:, 4:8, :], in_=qS[0, :, 4:8, :])
            nc.gpsimd.dma_start(out=pat_sb, in_=pat[:, :])
            for idx in (3, 4, 5, 6, 7):
                nc.gpsimd.dma_start(out=stiles[("q", idx)],
                                    in_=qS[idx, :, :, :])

            # ---- HAM warmup: prime the PE clock gate with back-to-back
            # garbage matmuls while the DMA head is in flight; also fire
            # one tiny Exp to pull the ACT spline tables in early.
            warm_act = singles.tile([128, 16], bf16, tag="warm_act")
            for w in range(12):
                wps = sps.tile([128, 1024], f32, tag="sp", name=f"warm{w}")
                nc.tensor.matmul(wps[:, 0:512], wrm[:, :128], wrm,
                                 start=True, stop=True)
                if w == 0:
                    nc.scalar.activation(
                        warm_act, wps[:, 0:16],
                        mybir.ActivationFunctionType.Exp, scale=0.001)

            fill_n = [0]

            def filler():
                def go():
                    fill_n[0] += 1
                    f = sps.tile([128, 1024], f32, tag="sp",
                                 name=f"fill{fill_n[0]}")
                    nc.tensor.matmul(f[:, 0:512], wrm[:, :128], wrm,
                                     start=True, stop=True)
                return go

            # ---- projection chains (pure compute; streams prefetched)
            def chain(kind, idx, split_first=False):
                w_sb = {"q": wq_sb, "k": wk_sb, "v": wv_sb}[kind]
                ph = pps.tile([64, 512], f32, tag="ph",
                              name=f"ph{kind}{idx}")
                col0 = idx * 512
                t = stiles[(kind, idx)]
                evs = []

                def step(c):
                    def go():
                        for i in range(4):
                            nc.tensor.matmul(
                                ph, w_sb[:, c + i, :], t[:, c + i, :],
                                start=(c + i == 0), stop=(c + i == DC - 1))
                    return go
                evs.extend(step(c) for c in range(0, DC, 4))

                def evict():
                    if kind == "q":
                        nc.vector.tensor_copy(qhT[0:64, col0:col0 + 512],
                                              ph)
                    elif kind == "k":
                        nc.vector.tensor_copy(khT[0:64, col0:col0 + 512], ph)
                    else:
                        vtmp = stream.tile([64, 512], bf16, tag="vtmp",
                                           bufs=2)
                        nc.vector.tensor_copy(vtmp, ph)
                        # PE identity-transpose [64h, 128pos] x4 into a
                        # borrowed S-pool slot, then copy into the PV
                        # weight layout (no DMA on the fold path).
                        vtp = sps.tile([128, 4, H], bf16, tag="sp",
                                       name=f"vtp{idx}")
                        for s in range(4):
                            nc.tensor.transpose(
                                vtp[:, s, :],
                                vtmp[:, s * 128:(s + 1) * 128], identb)
                        nc.vector.tensor_copy(
                            vh1[:, 4 * idx:4 * idx + 4, 0:H], vtp)
                evs.append(evict)
                return evs

            # ---- attention -------------------------------------------
            def s_pack(jj, m, nm, box):
                """S + exp for chunks (m, m+1) of pair jj; mask the
                pack that straddles the causal diagonal."""
                def go():
                    sp = sps.tile([128, 1024], f32, tag="sp",
                                  name=f"s{jj}_{m}")
                    jc = 512 * jj
                    for i in (0, 1):
                        nc.tensor.matmul(
                            sp[:, 512 * i:512 * i + 512],
                            khT[:, (m + i) * 128:(m + i + 1) * 128],
                            qhT[:, jc:jc + 512],
                            start=True, stop=True)
                    p = psbp.tile([128, 1024], bf16, tag="p")
                    nc.scalar.activation(
                        p, sp, mybir.ActivationFunctionType.Exp,
                        scale=scale)
                    if use_mask and m + 2 == nm:
                        nc.vector.tensor_mul(p, p, pat_sb)
                    box.append(p)
                return go

            def pv_pack(jj, m, nm, pvt, box):
                def go():
                    p = box[0]
                    for i in (0, 1):
                        nc.tensor.matmul(
                            pvt[:, :512], vh1[:, m + i, :],
                            p[:, 512 * i:512 * i + 512],
                            start=(m + i == 0), stop=(m + i == nm - 1),
                            skip_group_check=True)
                return go

            def drain(jj, pvt):
                def go():
                    dsb = stream.tile([H + 1, 512], bf16, tag="dsb",
                                      bufs=4)
                    nc.vector.tensor_copy(dsb, pvt)
                    for ps, pe in ((0, 33), (33, 65)):
                        nc.sync.dma_start(
                            out=outT[ps:pe, jj * 512:(jj + 1) * 512],
                            in_=dsb[ps:pe, :])
                return go

            def att_events(pairs, lag=2, fill=False):
                """Global software pipeline over all pairs: S-pack ...
                PV-pack (lagged), drain(jj) right after a pair's last
                PV.  fill=True slips a garbage matmul ahead of each
                S-pack so DMA waits never starve the HAM window."""
                evs = []
                pend = []  # (pv_event, after_event | None)

                def pop():
                    pv, after = pend.pop(0)
                    evs.append(pv)
                    if after is not None:
                        evs.append(after)

                for jj in pairs:
                    nm = n_chunks(jj)
                    pvt = pvp.tile([H + 1, 512], f32, tag="pv",
                                   name=f"pvt{jj}")
                    for m in range(0, nm, 2):
                        if fill:
                            evs.append(filler())
                        box = []
                        evs.append(s_pack(jj, m, nm, box))
                        last = (m + 2 == nm)
                        pend.append((pv_pack(jj, m, nm, pvt, box),
                                     drain(jj, pvt) if last else None))
                        if len(pend) > lag:
                            pop()
                while pend:
                    pop()
                return evs

            def with_fill(evs, n):
                return _merge(evs, [filler() for _ in range(n)])

            if use_mask:
                # head: kv group 0 + q pair 0, with fillers bridging the
                # DMA-latency head; then pairs with the remaining chains
                # merged in so the PE never runs dry.  Fillers between
                # chain steps bound every DMA wait to < the ~3.4us HAM
                # window so the clock gate stays open.
                head = (with_fill(_zip2(chain("k", 0, split_first=True),
                                        chain("v", 0)), 5)
                        + with_fill(chain("q", 0), 3))
                segs = [
                    ([0], with_fill(chain("q", 1), 4)),
                    ([1], with_fill(chain("k", 1) + chain("v", 1)
                                    + chain("q", 2), 6)),
                    ([2], with_fill(chain("q", 3), 3)),
                    ([3], with_fill(chain("k", 2) + chain("v", 2)
                                    + chain("q", 4), 4)),
                    ([4], chain("q", 5)),
                    ([5], chain("k", 3) + chain("v", 3) + chain("q", 6)),
                    ([6], chain("q", 7)),
                    ([7], []),
                ]
            else:
                head = (with_fill(_zip2(chain("k", 0, split_first=True),
                                        chain("v", 0)), 6)
                        + with_fill(_zip2(chain("k", 1) + chain("k", 2)
                                          + chain("k", 3),
                                          chain("v", 1) + chain("v", 2)
                                          + chain("v", 3)), 12)
                        + chain("q", 0))
                segs = [([jj], chain("q", jj + 1) if jj < 7 else [])
                        for jj in range(NPAIR)]

            for ev in head:
                ev()
            for pairs, chains in segs:
                for ev in _merge(chains, att_events(pairs)):
                    ev()
    nc.compile()
    return nc


def _get_program(key, use_mask):
    if key not in _CACHE:
        _CACHE[key] = _build_program(use_mask)
    return _CACHE[key]


def _numpy_fallback(q, k, v, mask, Wq, Wk, Wv):
    qh = q.astype(np.float32) @ Wq
    kh = k.astype(np.float32) @ Wk
    vh = v.astype(np.float32) @ Wv
    out = np.empty((B, T, H), np.float32)
    neg = np.float32(-1e30)
    for b in range(B):
        s = (qh[b] @ kh[b].T) / np.float32(np.sqrt(H))
        s = np.where(mask == 0, neg, s)
        s = s - s.max(axis=-1, keepdims=True)
        e = np.exp(s)
        w = e / e.sum(axis=-1, keepdims=True)
        out[b] = w @ vh[b]
    return out


def _w_layout(w, np_in):
    """[D, H] -> [128, DC, H]: partition-major layout for dense DMA."""
    return np.ascontiguousarray(
        w.reshape(DC, 128, H).transpose(1, 0, 2), np_in)


def _make_pat(half):
    """[128, 1024] stair mask for the diagonal pack: chunk m=2jj+h*?
    block i covers kv offsets 128*half + 256*i relative to the pair's
    first query column."""
    p = np.arange(128)[:, None]
    c = np.arange(512)[None, :]
    blocks = [(c >= 128 * half + 256 * i + p).astype(np.float32)
              for i in (0, 1)]
    return np.concatenate(blocks, axis=1)  # [128, 1024]


def _stream_pack(slabT, np_in):
    """[D, ncols] -> [ncols//512, 128, DC, 512] chain-major stream."""
    ncols = slabT.shape[1]
    return np.ascontiguousarray(
        slabT.reshape(DC, 128, ncols // 512, 512).transpose(2, 1, 0, 3),
        np_in)


def _make_in_maps(q, k, v, Wq, Wk, Wv, apply_tail, np_in):
    # core h owns global kv chunks 2m + h (128-position blocks)
    idx = [
        np.concatenate([np.arange(128 * (2 * m + h), 128 * (2 * m + h) + 128)
                        for m in range(NKVC)])
        for h in range(2)
    ]
    ones = np.ones((128, 1024), np.float32)
    in_maps = []
    for core in range(8):
        b, h = divmod(core, 2)
        im = {
            "qS": _stream_pack(q[b].T, np_in),
            "kS": _stream_pack(k[b].T[:, idx[h]], np_in),
            "vS": _stream_pack(v[b].T[:, idx[h]], np_in),
            "wq": _w_layout(Wq, np_in), "wk": _w_layout(Wk, np_in),
            "wv": _w_layout(Wv, np_in),
            "pat": np.ascontiguousarray(
                _make_pat(h) if apply_tail else ones, np_in),
        }
        in_maps.append(im)
    return in_maps


def kernel(q, k, v, mask, Wq, Wk, Wv):
    from concourse.bass_utils import run_bass_kernel_spmd
    import ml_dtypes

    q = np.ascontiguousarray(q, np.float32)
    k = np.ascontiguousarray(k, np.float32)
    v = np.ascontiguousarray(v, np.float32)
    Wq = np.ascontiguousarray(Wq, np.float32)
    Wk = np.ascontiguousarray(Wk, np.float32)
    Wv = np.ascontiguousarray(Wv, np.float32)
    mask = np.asarray(mask)

    is_tril = bool((mask == np.tril(np.ones((T, T), mask.dtype))).all())
    if not is_tril:
        return _numpy_fallback(q, k, v, mask, Wq, Wk, Wv)

    np_in = ml_dtypes.bfloat16
    nc = _get_program(("v27", is_tril), is_tril)

    in_maps = _make_in_maps(q, k, v, Wq, Wk, Wv, is_tril, np_in)
    res = run_bass_kernel_spmd(nc, in_maps, list(range(8)))

    out = np.empty((B, T, H), np.float32)
    for b in range(B):
        o0 = res.results[2 * b]["outT"].astype(np.float32)
        o1 = res.results[2 * b + 1]["outT"].astype(np.float32)
        num = o0[:H, :] + o1[:H, :]
        den = o0[H:H + 1, :] + o1[H:H + 1, :]
        out[b] = (num / den).T
    return out
